# revision 8
# baseline (speedup 1.0000x reference)
"""AuxSpatialGather (per-class masked mean pooling) Trainium2 kernel.

Computes, per sample b:  ctx[k, c] = mean over pixels n with gt[n]==k of feats[c, n]
(classes with zero pixels get 0), returned as [B, C, K, 1] float32.

Strategy (8 NeuronCores, data-parallel over batch, 2 samples/core):
  - The kernel is HBM-bound: 64 MiB of feats per core streams gapless on
    the SP HWDGE ring. Measured stream rate depends on DMA granularity
    (2 MB loads ~330 GB/s vs 0.5 MB ~306 GB/s), so loads are MIXED:
    2 MB [128ch, 4096px] granules for all chunks except the very last,
    which uses 0.5 MB quarter granules so the un-overlapped tail after
    the final HBM byte is one quarter's compute (~4 us) instead of a
    full chunk's (~17-27 us with HAM cold-clocking the PE).
  - fp32 matmul runs at 1/4 rate, so feats are cast f32->f16 (casts
    split DVE/ACT by channel granule), PE-transposed as PAIRS of f16
    pixels viewed as one f32 element (transpose-mode is a bit-exact raw
    mover), evacuated PSUM->SBUF (DVE/ACT alternating), and reduced by
    a one-hot matmul in f16 (two parity-split matmuls over a stride-2
    rhs view) with fp32 PSUM accumulation. Only precision loss: f16
    input quantization (~2e-4; fp8 measured 2.5e-2 - over tolerance).
  - pixel order n = qs*1024 + 8m + 2j + par: window (qs, j) reads
    stride-4 f32 columns of quarter qs only (a column slice of either a
    2 MB chunk tile or a 0.5 MB quarter tile), and the gt load lands in
    contiguous 32-byte runs on the second HWDGE ring (off the feat FIFO).
  - transposes are emitted ci-major within a quarter's 4 windows, so PE
    idle stays in slivers under the ~3.4 us HAM re-throttle window and
    the PE mostly holds its warm 2.4 GHz clock.
  - per-class counts via a free-dim reduce + ones-vector matmul; the
    final [19, 512] context is scaled by 1/max(cnt,1), transposed to
    [512, 19] on PE, and stored via SWDGE to keep the feat ring clean.
"""

import numpy as np

NUM_CLASSES = 19
B, C, H, W = 16, 512, 128, 128
HW = H * W
N_CORES = 8
S = B // N_CORES  # samples per core
P = 128  # partitions

_compiled = None


def _build_nc(s=S, c=C, hw=HW, cw=4096, qw=1024):
    from concourse import bacc, mybir
    from concourse.tile import TileContext
    from concourse.masks import make_identity

    f32 = mybir.dt.float32
    f16 = mybir.dt.float16
    i32 = mybir.dt.int32
    K = NUM_CLASSES
    n_ci = c // P  # channel granules (4)
    n_q = hw // cw  # chunks per sample (4)
    n_u = cw // qw  # quarters per chunk (4)
    n_g = hw // qw  # quarters per sample (16)
    n_w = 4  # windows (256 pixels) per quarter
    n_t = hw // P  # 128-pixel weight columns per sample (128)

    nc = bacc.Bacc("TRN2", target_bir_lowering=False)
    feats = nc.dram_tensor("feats", [s, c, hw], f32, kind="ExternalInput")
    gt = nc.dram_tensor("gt_seg_map", [s, hw], i32, kind="ExternalInput")
    out = nc.dram_tensor("out", [s, c, K], f32, kind="ExternalOutput")

    with TileContext(nc) as tc:
        with (
            tc.tile_pool(name="const", bufs=1) as const_pool,
            tc.tile_pool(name="stage", bufs=4) as stage_pool,
            tc.tile_pool(name="chunks", bufs=2) as ch_pool,
            tc.tile_pool(name="qstage", bufs=2) as qst_pool,
            tc.tile_pool(name="qchunk", bufs=2) as qch_pool,
            tc.tile_pool(name="planes", bufs=2) as plane_pool,
            tc.tile_pool(name="ft", bufs=4) as ft_pool,
            tc.tile_pool(name="small", bufs=2) as small_pool,
            tc.tile_pool(name="ftp", bufs=5, space="PSUM") as ftp_pool,
            tc.tile_pool(name="accp", bufs=2, space="PSUM") as acc_pool,
            tc.tile_pool(name="tinyp", bufs=1, space="PSUM") as tiny_pool,
        ):
            ident32 = const_pool.tile([P, P], f32)
            make_identity(nc, ident32[:])
            ones16 = const_pool.tile([P, 1], f16)
            nc.vector.memset(ones16[:], 1.0)

            # Pixel order: n = qs*qw + 8*m + 2*j + par
            # -> G[m, t], t = qs*8 + 2j + par: per-partition runs of 8
            # contiguous gt elements (32B); window (qs, j) reads stride-4
            # f32 pair-columns of quarter qs only.

            def load_chunk_big(si, q):
                """2 MB loads per granule (casts deferred to use time so
                this chunk's evacs aren't queued behind the next chunk's
                casts on DVE/ACT)."""
                sts = []
                for ci in range(n_ci):
                    st = stage_pool.tile([P, cw], f32, name="st")
                    nc.sync.dma_start(
                        out=st[:],
                        in_=feats[
                            si, ci * P : (ci + 1) * P, q * cw : (q + 1) * cw
                        ],
                    )
                    sts.append(st)
                return sts

            def cast_chunk_big(sts):
                chs = []
                for ci in range(n_ci):
                    ch = ch_pool.tile([P, cw], f16, name=f"ch{ci}")
                    if ci % 2 == 0:
                        nc.vector.tensor_copy(ch[:], sts[ci][:])
                    else:
                        nc.scalar.copy(ch[:], sts[ci][:])
                    chs.append(ch)
                return chs

            def load_quarter(si, qs):
                """0.5 MB loads for the final chunk (cast deferred to use)."""
                sts = []
                for ci in range(n_ci):
                    st = qst_pool.tile([P, qw], f32, name=f"stq{ci}")
                    nc.sync.dma_start(
                        out=st[:],
                        in_=feats[
                            si, ci * P : (ci + 1) * P, qs * qw : (qs + 1) * qw
                        ],
                    )
                    sts.append(st)
                return sts

            def cast_quarter(sts):
                chs = []
                for ci in range(n_ci):
                    ch = qch_pool.tile([P, qw], f16, name=f"chq{ci}")
                    if ci % 2 == 0:
                        nc.vector.tensor_copy(ch[:], sts[ci][:])
                    else:
                        nc.scalar.copy(ch[:], sts[ci][:])
                    chs.append(ch)
                return chs

            def build_planes(si):
                """One-hot planes for sample si (quarter-order pixel layout).
                gt DMA on the second HWDGE ring: off the feat FIFO."""
                G_i = plane_pool.tile([P, n_t], i32, name="G_i")
                nc.scalar.dma_start(
                    out=G_i[:].rearrange("p (qs r) -> p qs r", qs=n_g),
                    in_=gt[si].rearrange("(qs p r) -> p qs r", qs=n_g, p=P),
                )
                G_f = plane_pool.tile([P, n_t], f16, name="G_f")
                nc.vector.tensor_copy(G_f[:], G_i[:])
                planes = plane_pool.tile([P, K * n_t], f16, name="planes")
                for k in range(K):
                    nc.vector.tensor_scalar(
                        planes[:, k * n_t : (k + 1) * n_t],
                        G_f[:],
                        float(k),
                        None,
                        op0=mybir.AluOpType.is_equal,
                    )
                return planes

            def build_recip(planes):
                """Per-class counts -> reciprocal [K, 1]."""
                partial = small_pool.tile([P, K], f32, name="partial")
                nc.vector.tensor_reduce(
                    partial[:],
                    planes[:].rearrange("p (k t) -> p k t", k=K),
                    axis=mybir.AxisListType.X,
                    op=mybir.AluOpType.add,
                )
                partial16 = small_pool.tile([P, K], f16, name="partial16")
                nc.vector.tensor_copy(partial16[:], partial[:])
                cnt_ps = tiny_pool.tile([1, K], f32, name="cnt_ps", tag="tiny")
                nc.tensor.matmul(
                    cnt_ps[:], ones16[:], partial16[:], start=True, stop=True
                )
                cnt_sq = small_pool.tile([32, 32], f32, name="cnt_sq")
                nc.vector.memset(cnt_sq[:], 0.0)
                nc.vector.tensor_copy(cnt_sq[:1, :K], cnt_ps[:])
                cnt_tr = small_pool.tile([32, 32], f32, name="cnt_tr")
                nc.vector.transpose(cnt_tr[:], cnt_sq[:])
                recip = small_pool.tile([K, 1], f32, name="recip")
                nc.vector.tensor_scalar_max(recip[:], cnt_tr[:K, :1], 1.0)
                nc.vector.reciprocal(recip[:], recip[:])
                return recip

            # Feat loads own the SP ring and go first; gt + planes for
            # sample 0 follow on the ACT ring.
            pending_big = load_chunk_big(0, 0)
            planes_cur = build_planes(0)
            pending_q = None

            for si in range(s):
                for q in range(n_q):
                    is_final = si == s - 1 and q == n_q - 1
                    chs = None if is_final else cast_chunk_big(pending_big)
                    # prefetch the next chunk's loads
                    nsi, nq = (si, q + 1) if q + 1 < n_q else (si + 1, 0)
                    if nsi < s:
                        if nsi == s - 1 and nq == n_q - 1:
                            pending_q = [
                                load_quarter(nsi, nq * n_u + u)
                                for u in range(n_u)
                            ]
                        else:
                            pending_big = load_chunk_big(nsi, nq)
                    if q == 0:
                        acc = acc_pool.tile([K, c], f32, name="acc")
                        W_all = planes_cur[:].rearrange(
                            "p (k t) -> p t k", t=n_t
                        )
                        recip = build_recip(planes_cur)
                    if q == n_q - 2 and si + 1 < s:
                        planes_next = build_planes(si + 1)

                    for u in range(n_u):
                        qs = q * n_u + u
                        if is_final:
                            chs_u = cast_quarter(pending_q[u])
                            srcs = [chs_u[ci][:].bitcast(f32) for ci in range(n_ci)]
                            base = 0
                        else:
                            srcs = [chs[ci][:].bitcast(f32) for ci in range(n_ci)]
                            base = u * (qw // 2)
                        ftps = [
                            ftp_pool.tile([P, c], f32, name=f"ftp{j}", tag="ftp")
                            for j in range(n_w)
                        ]
                        for ci in range(n_ci):
                            for j in range(n_w):
                                nc.tensor.transpose(
                                    ftps[j][
                                        :, ci * P : (ci + 1) * P
                                    ],
                                    srcs[ci][
                                        :,
                                        base + j : base + j + (P - 1) * n_w + 1 : n_w,
                                    ],
                                    ident32[:],
                                )
                        for j in range(n_w):
                            fts = ft_pool.tile([P, 2 * c], f16, name="fts")
                            if j % 2 == 0:
                                nc.vector.tensor_copy(
                                    fts[:].bitcast(f32), ftps[j][:]
                                )
                            else:
                                nc.scalar.copy(fts[:].bitcast(f32), ftps[j][:])
                            fts_pairs = fts[:].rearrange(
                                "p (c two) -> p two c", two=2
                            )
                            for par in range(2):
                                t = qs * (2 * n_w) + 2 * j + par
                                nc.tensor.matmul(
                                    acc[:],
                                    W_all[:, t, :],
                                    fts_pairs[:, par, :],
                                    start=(t == 0),
                                    stop=(t == n_t - 1),
                                )

                # ---- normalize + emit [c, K] ----
                final = small_pool.tile([K, c], f32, name="final")
                nc.vector.tensor_scalar(
                    final[:], acc[:], recip[:, :1], None,
                    op0=mybir.AluOpType.mult,
                )
                outT_ps = tiny_pool.tile(
                    [P, n_ci * K], f32, name="outT_ps", tag="tiny"
                )
                for ci in range(n_ci):
                    nc.tensor.transpose(
                        outT_ps[:, ci * K : (ci + 1) * K],
                        final[:K, ci * P : (ci + 1) * P],
                        ident32[:K, :K],
                    )
                outT = small_pool.tile([P, n_ci * K], f32, name="outT")
                nc.vector.tensor_copy(outT[:], outT_ps[:])
                # SWDGE: keep the HWDGE feat-load queue free of DMAs that
                # wait on compute (FIFO per issuing engine)
                nc.gpsimd.dma_start(
                    out=out[si].rearrange("(ci p) k -> p ci k", p=P),
                    in_=outT[:].rearrange("p (ci k) -> p ci k", k=K),
                )
                if si + 1 < s:
                    planes_cur = planes_next
    nc.compile()
    return nc


def _get_compiled():
    global _compiled
    if _compiled is None:
        _compiled = _build_nc()
    return _compiled


def kernel(feats, gt_seg_map):
    from concourse.bass_utils import run_bass_kernel_spmd

    feats = np.asarray(feats, dtype=np.float32).reshape(B, C, HW)
    gt = np.asarray(gt_seg_map).astype(np.int32).reshape(B, HW)

    nc = _get_compiled()
    in_maps = []
    for i in range(N_CORES):
        in_maps.append(
            {
                "feats": feats[i * S : (i + 1) * S],
                "gt_seg_map": gt[i * S : (i + 1) * S],
            }
        )
    res = run_bass_kernel_spmd(nc, in_maps, core_ids=list(range(N_CORES)))
    parts = [res.results[i]["out"] for i in range(N_CORES)]  # each [S, C, K]
    full = np.concatenate(parts, axis=0)  # [B, C, K]
    return full[..., None].astype(np.float32)  # [B, C, K, 1]


# revision 10
# speedup vs baseline: 1.0259x; 1.0259x over previous
"""AuxSpatialGather (per-class masked mean pooling) Trainium2 kernel.

Computes, per sample b:  ctx[k, c] = mean over pixels n with gt[n]==k of feats[c, n]
(classes with zero pixels get 0), returned as [B, C, K, 1] float32.

Strategy (8 NeuronCores, data-parallel over batch, 2 samples/core):
  - The kernel is HBM-bound: 64 MiB of feats per core streams gapless on
    the SP HWDGE ring. Measured stream rate depends on DMA granularity
    (2 MB loads ~330 GB/s vs 0.5 MB ~306 GB/s), so loads are MIXED:
    2 MB [128ch, 4096px] granules for all chunks except the very last,
    which uses 0.5 MB quarter granules so the un-overlapped tail after
    the final HBM byte is one quarter's compute (~4 us) instead of a
    full chunk's (~17-27 us with HAM cold-clocking the PE).
  - fp32 matmul runs at 1/4 rate, so feats are cast f32->f16 (casts
    split DVE/ACT by channel granule), PE-transposed as PAIRS of f16
    pixels viewed as one f32 element (transpose-mode is a bit-exact raw
    mover), evacuated PSUM->SBUF (DVE/ACT alternating), and reduced by
    a one-hot matmul in f16 (two parity-split matmuls over a stride-2
    rhs view) with fp32 PSUM accumulation. Only precision loss: f16
    input quantization (~2e-4; fp8 measured 2.5e-2 - over tolerance).
  - pixel order n = qs*1024 + 8m + 2j + par: window (qs, j) reads
    stride-4 f32 columns of quarter qs only (a column slice of either a
    2 MB chunk tile or a 0.5 MB quarter tile), and the gt load lands in
    contiguous 32-byte runs on the second HWDGE ring (off the feat FIFO).
  - transposes are emitted ci-major within a quarter's 4 windows, so PE
    idle stays in slivers under the ~3.4 us HAM re-throttle window and
    the PE mostly holds its warm 2.4 GHz clock.
  - per-class counts via a free-dim reduce + ones-vector matmul; the
    final [19, 512] context is scaled by 1/max(cnt,1), transposed to
    [512, 19] on PE, and stored via SWDGE to keep the feat ring clean.
"""

import numpy as np

NUM_CLASSES = 19
B, C, H, W = 16, 512, 128, 128
HW = H * W
N_CORES = 8
S = B // N_CORES  # samples per core
P = 128  # partitions

_compiled = None


def _build_nc(s=S, c=C, hw=HW, cw=4096, qw=1024):
    from concourse import bacc, mybir
    from concourse.tile import TileContext
    from concourse.masks import make_identity

    f32 = mybir.dt.float32
    f16 = mybir.dt.float16
    i32 = mybir.dt.int32
    K = NUM_CLASSES
    n_ci = c // P  # channel granules (4)
    n_q = hw // cw  # chunks per sample (4)
    n_u = cw // qw  # quarters per chunk (4)
    n_g = hw // qw  # quarters per sample (16)
    n_w = 4  # windows (256 pixels) per quarter
    n_t = hw // P  # 128-pixel weight columns per sample (128)

    nc = bacc.Bacc("TRN2", target_bir_lowering=False)
    feats = nc.dram_tensor("feats", [s, c, hw], f32, kind="ExternalInput")
    gt = nc.dram_tensor("gt_seg_map", [s, hw], i32, kind="ExternalInput")
    out = nc.dram_tensor("out", [s, c, K], f32, kind="ExternalOutput")

    with TileContext(nc) as tc:
        with (
            tc.tile_pool(name="const", bufs=1) as const_pool,
            tc.tile_pool(name="stage", bufs=4) as stage_pool,
            tc.tile_pool(name="chunks", bufs=2) as ch_pool,
            tc.tile_pool(name="qstage", bufs=2) as qst_pool,
            tc.tile_pool(name="qchunk", bufs=2) as qch_pool,
            tc.tile_pool(name="planes", bufs=2) as plane_pool,
            tc.tile_pool(name="ft", bufs=4) as ft_pool,
            tc.tile_pool(name="small", bufs=2) as small_pool,
            tc.tile_pool(name="ftp", bufs=5, space="PSUM") as ftp_pool,
            tc.tile_pool(name="accp", bufs=2, space="PSUM") as acc_pool,
            tc.tile_pool(name="tinyp", bufs=1, space="PSUM") as tiny_pool,
        ):
            ident32 = const_pool.tile([P, P], f32)
            make_identity(nc, ident32[:])
            ones16 = const_pool.tile([P, 1], f16)
            nc.vector.memset(ones16[:], 1.0)

            # Pixel order: n = qs*qw + 8*m + 2*j + par
            # -> G[m, t], t = qs*8 + 2j + par: per-partition runs of 8
            # contiguous gt elements (32B); window (qs, j) reads stride-4
            # f32 pair-columns of quarter qs only.

            def load_chunk_big(si, q):
                """2 MB loads per granule (casts deferred to use time so
                this chunk's evacs aren't queued behind the next chunk's
                casts on DVE/ACT)."""
                sts = []
                for ci in range(n_ci):
                    st = stage_pool.tile([P, cw], f32, name="st")
                    nc.sync.dma_start(
                        out=st[:],
                        in_=feats[
                            si, ci * P : (ci + 1) * P, q * cw : (q + 1) * cw
                        ],
                    )
                    sts.append(st)
                return sts

            def cast_chunk_big(sts):
                chs = []
                for ci in range(n_ci):
                    ch = ch_pool.tile([P, cw], f16, name=f"ch{ci}")
                    if ci % 2 == 0:
                        nc.vector.tensor_copy(ch[:], sts[ci][:])
                    else:
                        nc.scalar.copy(ch[:], sts[ci][:])
                    chs.append(ch)
                return chs

            def load_quarter(si, qs):
                """0.5 MB loads for the final chunk (cast deferred to use)."""
                sts = []
                for ci in range(n_ci):
                    st = qst_pool.tile([P, qw], f32, name=f"stq{ci}")
                    nc.sync.dma_start(
                        out=st[:],
                        in_=feats[
                            si, ci * P : (ci + 1) * P, qs * qw : (qs + 1) * qw
                        ],
                    )
                    sts.append(st)
                return sts

            def cast_quarter(sts):
                chs = []
                for ci in range(n_ci):
                    ch = qch_pool.tile([P, qw], f16, name=f"chq{ci}")
                    if ci % 2 == 0:
                        nc.vector.tensor_copy(ch[:], sts[ci][:])
                    else:
                        nc.scalar.copy(ch[:], sts[ci][:])
                    chs.append(ch)
                return chs

            def build_planes(si):
                """One-hot planes for sample si (quarter-order pixel layout).
                gt DMA on the second HWDGE ring: off the feat FIFO."""
                G_i = plane_pool.tile([P, n_t], i32, name="G_i")
                nc.scalar.dma_start(
                    out=G_i[:].rearrange("p (qs r) -> p qs r", qs=n_g),
                    in_=gt[si].rearrange("(qs p r) -> p qs r", qs=n_g, p=P),
                )
                G_f = plane_pool.tile([P, n_t], f16, name="G_f")
                nc.vector.tensor_copy(G_f[:], G_i[:])
                planes = plane_pool.tile([P, K * n_t], f16, name="planes")
                for k in range(K):
                    nc.vector.tensor_scalar(
                        planes[:, k * n_t : (k + 1) * n_t],
                        G_f[:],
                        float(k),
                        None,
                        op0=mybir.AluOpType.is_equal,
                    )
                return planes

            def build_recip(planes):
                """Per-class counts -> reciprocal [K, 1]."""
                partial = small_pool.tile([P, K], f32, name="partial")
                nc.vector.tensor_reduce(
                    partial[:],
                    planes[:].rearrange("p (k t) -> p k t", k=K),
                    axis=mybir.AxisListType.X,
                    op=mybir.AluOpType.add,
                )
                partial16 = small_pool.tile([P, K], f16, name="partial16")
                nc.vector.tensor_copy(partial16[:], partial[:])
                cnt_ps = tiny_pool.tile([1, K], f32, name="cnt_ps", tag="tiny")
                nc.tensor.matmul(
                    cnt_ps[:], ones16[:], partial16[:], start=True, stop=True
                )
                cnt_sq = small_pool.tile([32, 32], f32, name="cnt_sq")
                nc.vector.memset(cnt_sq[:], 0.0)
                nc.vector.tensor_copy(cnt_sq[:1, :K], cnt_ps[:])
                cnt_tr = small_pool.tile([32, 32], f32, name="cnt_tr")
                nc.vector.transpose(cnt_tr[:], cnt_sq[:])
                recip = small_pool.tile([K, 1], f32, name="recip")
                nc.vector.tensor_scalar_max(recip[:], cnt_tr[:K, :1], 1.0)
                nc.vector.reciprocal(recip[:], recip[:])
                return recip

            # Feat loads own the SP ring and go first; gt + planes for
            # sample 0 follow on the ACT ring.
            pending_big = load_chunk_big(0, 0)
            planes_cur = build_planes(0)
            pending_q = None

            for si in range(s):
                for q in range(n_q):
                    is_final = si == s - 1 and q == n_q - 1
                    chs = None if is_final else cast_chunk_big(pending_big)
                    # prefetch the next chunk's loads
                    nsi, nq = (si, q + 1) if q + 1 < n_q else (si + 1, 0)
                    if nsi < s:
                        if nsi == s - 1 and nq == n_q - 1:
                            pending_q = [
                                load_quarter(nsi, nq * n_u + u)
                                for u in range(n_u)
                            ]
                        else:
                            pending_big = load_chunk_big(nsi, nq)
                    if q == 0:
                        acc = acc_pool.tile([K, c], f32, name="acc")
                        W_all = planes_cur[:].rearrange(
                            "p (k t) -> p t k", t=n_t
                        )
                        recip = build_recip(planes_cur)

                    for u in range(n_u):
                        qs = q * n_u + u
                        if is_final:
                            chs_u = cast_quarter(pending_q[u])
                            srcs = [chs_u[ci][:].bitcast(f32) for ci in range(n_ci)]
                            base = 0
                        else:
                            srcs = [chs[ci][:].bitcast(f32) for ci in range(n_ci)]
                            base = u * (qw // 2)
                        ftps = [
                            ftp_pool.tile([P, c], f32, name=f"ftp{j}", tag="ftp")
                            for j in range(n_w)
                        ]
                        for ci in range(n_ci):
                            for j in range(n_w):
                                nc.tensor.transpose(
                                    ftps[j][
                                        :, ci * P : (ci + 1) * P
                                    ],
                                    srcs[ci][
                                        :,
                                        base + j : base + j + (P - 1) * n_w + 1 : n_w,
                                    ],
                                    ident32[:],
                                )
                        for j in range(n_w):
                            fts = ft_pool.tile([P, 2 * c], f16, name="fts")
                            if j % 2 == 0:
                                nc.vector.tensor_copy(
                                    fts[:].bitcast(f32), ftps[j][:]
                                )
                            else:
                                nc.scalar.copy(fts[:].bitcast(f32), ftps[j][:])
                            fts_pairs = fts[:].rearrange(
                                "p (c two) -> p two c", two=2
                            )
                            for par in range(2):
                                t = qs * (2 * n_w) + 2 * j + par
                                nc.tensor.matmul(
                                    acc[:],
                                    W_all[:, t, :],
                                    fts_pairs[:, par, :],
                                    start=(t == 0),
                                    stop=(t == n_t - 1),
                                )

                    # prefetch next sample's gt + planes AFTER this chunk's
                    # evacs, so the G_f cast (gated on the gt DMA) doesn't
                    # head-of-line-block them on DVE
                    if q == n_q - 2 and si + 1 < s:
                        planes_next = build_planes(si + 1)

                # ---- normalize + emit [c, K] ----
                final = small_pool.tile([K, c], f32, name="final")
                nc.vector.tensor_scalar(
                    final[:], acc[:], recip[:, :1], None,
                    op0=mybir.AluOpType.mult,
                )
                outT_ps = tiny_pool.tile(
                    [P, n_ci * K], f32, name="outT_ps", tag="tiny"
                )
                for ci in range(n_ci):
                    nc.tensor.transpose(
                        outT_ps[:, ci * K : (ci + 1) * K],
                        final[:K, ci * P : (ci + 1) * P],
                        ident32[:K, :K],
                    )
                outT = small_pool.tile([P, n_ci * K], f32, name="outT")
                nc.vector.tensor_copy(outT[:], outT_ps[:])
                # SWDGE: keep the HWDGE feat-load queue free of DMAs that
                # wait on compute (FIFO per issuing engine)
                nc.gpsimd.dma_start(
                    out=out[si].rearrange("(ci p) k -> p ci k", p=P),
                    in_=outT[:].rearrange("p (ci k) -> p ci k", k=K),
                )
                if si + 1 < s:
                    planes_cur = planes_next
    nc.compile()
    return nc


def _get_compiled():
    global _compiled
    if _compiled is None:
        _compiled = _build_nc()
    return _compiled


def kernel(feats, gt_seg_map):
    from concourse.bass_utils import run_bass_kernel_spmd

    feats = np.asarray(feats, dtype=np.float32).reshape(B, C, HW)
    gt = np.asarray(gt_seg_map).astype(np.int32).reshape(B, HW)

    nc = _get_compiled()
    in_maps = []
    for i in range(N_CORES):
        in_maps.append(
            {
                "feats": feats[i * S : (i + 1) * S],
                "gt_seg_map": gt[i * S : (i + 1) * S],
            }
        )
    res = run_bass_kernel_spmd(nc, in_maps, core_ids=list(range(N_CORES)))
    parts = [res.results[i]["out"] for i in range(N_CORES)]  # each [S, C, K]
    full = np.concatenate(parts, axis=0)  # [B, C, K]
    return full[..., None].astype(np.float32)  # [B, C, K, 1]
